# revision 11
# baseline (speedup 1.0000x reference)
"""FocalLoss + MDCA loss kernel for TRN2, 8-core data-parallel.

reference:
    loss_cls = mean_i[-(1-pt_i) * log(pt_i)],  pt_i = probs[i, targets[i]]
    loss_cal = mean_c |mean_i probs[i,c] - count_c/B|
    out = loss_cls + loss_cal        (GAMMA=1, BETA=1)

Strategy (v3): shard batch (16384) across 8 cores (2048 rows each).
The per-core floor is streaming the 8.2 MB fp32 probs shard from HBM;
all other compute hides under that stream.

  - probs stream HBM->SBUF as plain 4-byte copies via 16 HWDGE
    dma_starts on the SP ring. The DRAM tensor and SBUF tiles are
    declared float32r so the PE streams them at 1 cycle/row (fp32
    proper is 4 cycles/row; an f32->f32r bitcast fails BIR
    verification, so the dtype is f32r end-to-end and exact-f32
    consumers bitcast back).
  - colsum: 32 accumulating [1,500] matmuls with a ones_f32r weight.
  - histogram: DVE builds eqsum[p,c] = sum_i (c == target_{i,p}) as a
    16-link fused is_equal+add chain (fp16, 2x mode) over a host-sent
    iota row, then ONE matmul pair with ones_f16 yields the exact
    histogram - 2 PE matmuls instead of 32 (PE was the pacer in v2).
  - targets arrive host-prepacked as aux [128, 33] f32r:
    [t_cols f32 | gather offsets i32 | ones f32r], one tiny DMA.
  - pt[p,j] = probs[128j+p, t] via one SWDGE indirect gather (exact
    fp32); focal partials (pt-1)*ln(pt) reduce on ACT+DVE and are
    partition-reduced by a tiny exact fp32 matmul with ones.
  - hist+focal drain early into their own DMA; the final DMA carries
    only the 4 KB colsum row, so the tail after the last tile is
    colsum_15 -> two parallel PSUM drains -> 4 KB store.
Host combines the 8 cores' colsum/hist/focal partials into the scalar.

The walrus build in this env encodes at most ONE sync wait per
instruction; _split_multi_waits post-processes the scheduled program to
hoist extra waits onto same-engine EventSemaphore carriers.
"""

import numpy as np

import concourse.bass as bass
import concourse.mybir as mybir
import concourse.tile as tile
from concourse.bass_utils import run_bass_kernel_spmd

B, C = 16384, 1000
NCORES = 8
BC = B // NCORES  # 2048 rows per core
P = 128
NT = BC // P      # 16 batch tiles per core
CH = 500          # matmul half free-dim (PSUM bank = 512 fp32)
OUT_W = 2001      # [colsum 0:1000 | hist 1000:2000 | focal_sum 2000]
AUXW = 2 * NT + 1  # [t_cols 0:16 | offs 16:32 | ones 32]

F32 = mybir.dt.float32
FR = mybir.dt.float32r
F16 = mybir.dt.float16
I32 = mybir.dt.int32


def emit_kernel(ctx, tc, probs_d, aux_d, iota_d, out_hf_d, out_cs_d):
    nc = tc.nc
    Alu = mybir.AluOpType

    consts = ctx.enter_context(tc.tile_pool(name="consts", bufs=1))
    probs_pool = ctx.enter_context(tc.tile_pool(name="probs_pool", bufs=NT))
    eq_pool = ctx.enter_context(tc.tile_pool(name="eq_pool", bufs=2))
    psum = ctx.enter_context(tc.tile_pool(name="psum", bufs=1, space="PSUM"))

    # 1) probs loads: plain 4-byte HWDGE on the SP ring, first in program
    # order so the SDMA engines stream continuously.
    pf32s = []
    for i in range(NT):
        pf = probs_pool.tile([P, C], FR, tag="pf32", name=f"pf32_{i}")
        nc.sync.dma_start(out=pf[:], in_=probs_d[i * P:(i + 1) * P, :])
        pf32s.append(pf)

    # 2) small inputs on the ACT ring: aux (targets/offsets/ones), iota row.
    aux = consts.tile([P, AUXW], FR, tag="aux")
    nc.scalar.dma_start(out=aux[:], in_=aux_d[:, :])
    t_cols = aux[:, 0:NT].bitcast(F32)
    offs = aux[:, NT:2 * NT].bitcast(I32)
    ones_fr = aux[:, 2 * NT:2 * NT + 1]

    iota_f16 = consts.tile([P, C], F16, tag="iota_f16")
    nc.scalar.dma_start(out=iota_f16[:], in_=iota_d[:, :])

    ones_f32 = consts.tile([P, 1], F32, tag="ones_f32")
    nc.vector.memset(ones_f32[:], 1.0)
    ones_f16 = consts.tile([P, 1], F16, tag="ones_f16")
    nc.vector.memset(ones_f16[:], 1.0)

    # 3) pt[p, j] = probs[128j + p, t] in ONE indirect gather (exact fp32)
    # on the otherwise-idle SWDGE path.
    pt_all = consts.tile([P, NT], F32, tag="pt_all")
    nc.gpsimd.indirect_dma_start(
        out=pt_all[:], out_offset=None,
        in_=probs_d.rearrange("a b -> (a b)").bitcast(F32)[:, None],
        in_offset=bass.IndirectOffsetOnAxis(ap=offs, axis=0),
    )

    # 4) eqsum chain on DVE: acc_i = (iota == t_i) + acc_{i-1}, fp16
    # (counts <= 16 are exact). Ping-pong between two buffers.
    acc = [eq_pool.tile([P, C], F16, tag="eq", name=f"eq_{j}") for j in range(2)]
    nc.vector.tensor_scalar(
        out=acc[0][:], in0=iota_f16[:], scalar1=t_cols[:, 0:1],
        scalar2=None, op0=Alu.is_equal,
    )
    for i in range(1, NT):
        nc.vector.scalar_tensor_tensor(
            out=acc[i % 2][:], in0=iota_f16[:], scalar=t_cols[:, i:i + 1],
            in1=acc[(i - 1) % 2][:], op0=Alu.is_equal, op1=Alu.add,
        )
    eqsum = acc[(NT - 1) % 2]

    # 5) focal partials: pl = [pt | ln(pt)] via ACT, then DVE fuses
    # (pt-1)*ln(pt) with a row-sum into focal[128, 1].
    pl = consts.tile([P, 2 * NT], F32, tag="pl")
    nc.scalar.copy(pl[:, 0:NT], pt_all[:])
    nc.scalar.activation(pl[:, NT:2 * NT], pt_all[:],
                         mybir.ActivationFunctionType.Ln)
    junk = consts.tile([P, NT], F32, tag="junk")
    focal = consts.tile([P, 1], F32, tag="focal")
    nc.vector.scalar_tensor_tensor(
        out=junk[:], in0=pl[:, 0:NT], scalar=1.0, in1=pl[:, NT:2 * NT],
        op0=Alu.subtract, op1=Alu.mult, accum_out=focal[:],
    )

    # 6) PSUM accumulators (5 banks: cs x2, hs x2, fc).
    cs_ps = [psum.tile([1, CH], F32, tag=f"cs_ps{h}", name=f"cs_ps{h}")
             for h in range(2)]
    hs_ps = [psum.tile([1, CH], F32, tag=f"hs_ps{h}", name=f"hs_ps{h}")
             for h in range(2)]
    fc_ps = psum.tile([1, 1], F32, tag="fc_ps")

    def colsum(i):
        for h in range(2):
            sl = slice(h * CH, (h + 1) * CH)
            nc.tensor.matmul(cs_ps[h][:], ones_fr, pf32s[i][:, sl],
                             start=(i == 0), stop=(i == NT - 1))

    for i in range(NT - 1):
        colsum(i)
        if i == 2:
            # exact fp32 partition-reduction of the focal partials
            nc.tensor.matmul(fc_ps[:], ones_f32[:], focal[:],
                             start=True, stop=True)
    for h in range(2):
        sl = slice(h * CH, (h + 1) * CH)
        nc.tensor.matmul(hs_ps[h][:], ones_f16[:], eqsum[:, sl],
                         start=True, stop=True)
    colsum(NT - 1)

    # 7) hist + focal drain early on ACT into their own small DMA.
    out_hf = consts.tile([1, C + 1], F32, tag="out_hf")
    for h in range(2):
        nc.scalar.copy(out_hf[:, h * CH:(h + 1) * CH], hs_ps[h][:])
    nc.scalar.copy(out_hf[:, C:C + 1], fc_ps[:])
    nc.scalar.dma_start(out=out_hf_d[:, :], in_=out_hf[:])

    # 8) colsum tail: two parallel PSUM drains (DVE h0, ACT h1), 4 KB store.
    out_cs = consts.tile([1, C], F32, tag="out_cs")
    nc.vector.tensor_copy(out_cs[:, 0:CH], cs_ps[0][:])
    nc.scalar.copy(out_cs[:, CH:C], cs_ps[1][:])
    nc.sync.dma_start(out=out_cs_d[:, :], in_=out_cs[:])


def _split_multi_waits(nc):
    """The walrus build in this env encodes at most ONE sync wait per
    instruction (newer Tile emits several, e.g. on its tail drain). Hoist
    extra waits onto EventSemaphore carrier instructions inserted just
    before, on the same engine — same-engine program order makes this
    semantically identical."""
    n = 0
    for f in nc.m.functions:
        for blk in f.blocks:
            il = blk.instructions
            i = 0
            while i < len(il):
                inst = il[i]
                si = inst.sync_info
                ws = list(si.on_wait) if si is not None else []
                if len(ws) > 1:
                    for w in ws[:-1]:
                        ev = mybir.InstEventSemaphore(
                            name=f"I-waitsplit-{n}", ins=[], outs=[])
                        n += 1
                        ev.engine = inst.engine
                        ev.sync_info = mybir.SyncInfo(on_wait=[w], on_update=[])
                        il.insert(i, ev)
                        i += 1
                    inst.sync_info = mybir.SyncInfo(
                        on_wait=[ws[-1]], on_update=list(si.on_update))
                i += 1


_cached_nc = {}


def build_nc(split_waits=True):
    global _cached_nc
    if split_waits in _cached_nc:
        return _cached_nc[split_waits]
    from contextlib import ExitStack

    nc = bass.Bass("TRN2", dynamic_dma_scratch_size=65536)
    probs_d = nc.dram_tensor("probs", [BC, C], FR, kind="ExternalInput").ap()
    aux_d = nc.dram_tensor("aux", [P, AUXW], FR, kind="ExternalInput").ap()
    iota_d = nc.dram_tensor("iota16", [P, C], F16, kind="ExternalInput").ap()
    out_hf_d = nc.dram_tensor("out_hf", [1, C + 1], F32,
                              kind="ExternalOutput").ap()
    out_cs_d = nc.dram_tensor("out_cs", [1, C], F32, kind="ExternalOutput").ap()

    with tile.TileContext(nc) as tc:
        with ExitStack() as ctx:
            emit_kernel(ctx, tc, probs_d, aux_d, iota_d, out_hf_d, out_cs_d)
    if split_waits:
        _split_multi_waits(nc)
    _cached_nc[split_waits] = nc
    return nc


def make_in_maps(probs, targets):
    probs = np.ascontiguousarray(np.asarray(probs), dtype=np.float32)
    targets = np.asarray(targets).astype(np.int64)
    assert probs.shape == (B, C) and targets.shape == (B,)
    rows = np.arange(NT)[None, :] * P + np.arange(P)[:, None]  # [128, 16]
    iota16 = np.broadcast_to(np.arange(C, dtype=np.float16), (P, C)).copy()
    maps = []
    for k in range(NCORES):
        t = targets[k * BC:(k + 1) * BC].reshape(NT, P).T     # [128, 16]
        aux = np.empty((P, AUXW), np.float32)
        aux[:, :NT] = t.astype(np.float32)
        aux[:, NT:2 * NT] = (rows * C + t).astype(np.int32).view(np.float32)
        aux[:, 2 * NT] = 1.0
        maps.append({"probs": probs[k * BC:(k + 1) * BC], "aux": aux,
                     "iota16": iota16})
    return maps


def combine(results):
    cs = np.zeros(C, np.float64)
    hs = np.zeros(C, np.float64)
    fc = 0.0
    for r in results:
        cs += r["out_cs"].reshape(C).astype(np.float64)
        hf = r["out_hf"].reshape(C + 1).astype(np.float64)
        hs += hf[0:C]
        fc += hf[C]
    loss_cls = fc / B
    loss_cal = float(np.mean(np.abs(cs / B - hs / B)))
    return np.asarray(loss_cls + 1.0 * loss_cal, dtype=np.float32)


def run_spmd(probs, targets, **kwargs):
    nc = build_nc()
    in_maps = make_in_maps(probs, targets)
    return run_bass_kernel_spmd(nc, in_maps, list(range(NCORES)), **kwargs)


def kernel(probs, targets):
    res = run_spmd(probs, targets)
    return combine(res.results)
